# revision 4
# baseline (speedup 1.0000x reference)
"""Trainium2 Bass kernel for a cached-encoder-layer block.

Computation (per batch b):
    S  = (x_b @ x_b^T) * scale          # single-head scores, scale=(D//n_head)^-0.5
    P  = softmax(S, axis=-1)
    a  = P @ x_b
    h  = LN(a + x_b) * gamma1 + beta1   # LayerNorm over D
    f  = relu(h @ W1 + b1) @ W2 + b2
    out= LN(f + h) * gamma2 + beta2

Sharding: 8 cores = (batch b in 0..3) x (query-half in 0..1). Each core gets
its batch's keys/values rotated so its 2048 queries are rows 0..2047; softmax
is permutation-invariant over keys so the rotation is harmless and makes the
program identical (SPMD) on all cores.

Per-core kernel strategy:
  - scores are computed TRANSPOSED: ST[k, q] tiles, so that after exp() the
    probability tiles are directly the lhsT (stationary) operand of the PV
    matmul - no on-device transposes in the attention path.
  - softmax is computed without max-subtraction (safe: scores*scale <= ~70
    for randn-scale inputs, well within fp32 exp range); the row-sum comes
    for free as a 257th "ones" column appended to the value matrix.
  - matmul operands are held in MM_DT (bf16 default, fp32r optional); all
    accumulation is fp32 in PSUM; layernorm/softmax statistics are fp32.
  - FFN computes relu(h@W1+b1) transposed (f1T[h, q]) so b1 is a
    per-partition activation bias and f1T feeds FFN2 as lhsT directly.
"""

import os

import ml_dtypes
import numpy as np

import concourse.bacc as bacc
import concourse.bass as bass
import concourse.mybir as mybir
import concourse.tile as tile
from concourse.bass_utils import run_bass_kernel_spmd
from concourse.masks import make_identity

B, S, D, H = 4, 4096, 256, 1024
NCORES = 8
SQ = S // 2            # queries per core
QS = 256               # q-strip width
NSTRIP = SQ // QS      # 8
NKC = S // 128         # 32 key chunks
NQT = SQ // 128        # 16 q tiles per core
F32 = mybir.dt.float32
AF = mybir.ActivationFunctionType
ALU = mybir.AluOpType

if os.environ.get("MM_DT", "bf16") == "f32r":
    MM_DT = mybir.dt.float32r
    MM_NP = np.float32
else:
    MM_DT = mybir.dt.bfloat16
    MM_NP = ml_dtypes.bfloat16


def build_program(scale: float, use_gb1: bool, use_gb2: bool, use_b2: bool):
    nc = bacc.Bacc(trn_type="TRN2")

    xkT_d = nc.dram_tensor("xkT", [D, S], MM_DT, kind="ExternalInput")
    xv_d = nc.dram_tensor("xv", [S, D + 1], MM_DT, kind="ExternalInput")
    xq_d = nc.dram_tensor("xq", [SQ, D], F32, kind="ExternalInput")
    w1_d = nc.dram_tensor("w1", [D, H], MM_DT, kind="ExternalInput")
    w2_d = nc.dram_tensor("w2", [H, D], MM_DT, kind="ExternalInput")
    b1_d = nc.dram_tensor("b1", [H], F32, kind="ExternalInput")
    if use_b2:
        b2_d = nc.dram_tensor("b2", [D], F32, kind="ExternalInput")
    if use_gb1:
        g1_d = nc.dram_tensor("g1", [D], F32, kind="ExternalInput")
        bt1_d = nc.dram_tensor("bt1", [D], F32, kind="ExternalInput")
    if use_gb2:
        g2_d = nc.dram_tensor("g2", [D], F32, kind="ExternalInput")
        bt2_d = nc.dram_tensor("bt2", [D], F32, kind="ExternalInput")
    out_d = nc.dram_tensor("out", [SQ, D], F32, kind="ExternalOutput")

    def bcast_row(ap_1d, parts=128):
        # [N] dram vector -> [[0,parts],[1,N]] AP (same row in every partition)
        return bass.AP(
            tensor=ap_1d.tensor, offset=ap_1d.offset, ap=[[0, parts], ap_1d.ap[0]]
        )

    with (
        tile.TileContext(nc) as tc,
        tc.tile_pool(name="const", bufs=1) as constp,
        tc.tile_pool(name="ptp", bufs=44) as ptp,
        tc.tile_pool(name="hall", bufs=1) as hallp,
        tc.tile_pool(name="htp", bufs=3) as htp,
        tc.tile_pool(name="f1p", bufs=2) as f1p,
        tc.tile_pool(name="workp", bufs=4) as workp,
        tc.tile_pool(name="statp", bufs=8) as statp,
        tc.tile_pool(name="ps_st", bufs=2, space="PSUM") as ps_st,
        tc.tile_pool(name="ps_o", bufs=2, space="PSUM") as ps_o,
        tc.tile_pool(name="ps_f1", bufs=2, space="PSUM") as ps_f1,
        tc.tile_pool(name="ps_misc", bufs=2, space="PSUM") as ps_misc,
    ):
        # ---------------- resident inputs ----------------
        xkT_sb = constp.tile([128, 2, S], MM_DT, name="xkT_sb")
        xkT_r = xkT_d.rearrange("(dc p) k -> p dc k", p=128)
        for i in range(8):
            sl = slice(i * (S // 8), (i + 1) * (S // 8))
            nc.sync.dma_start(out=xkT_sb[:, :, sl], in_=xkT_r[:, :, sl])

        xv_sb = constp.tile([128, NKC, D + 1], MM_DT, name="xv_sb")
        xv_r = xv_d.rearrange("(n p) c -> p n c", p=128)
        for i in range(8):
            sl = slice(i * (NKC // 8), (i + 1) * (NKC // 8))
            nc.sync.dma_start(out=xv_sb[:, sl, :], in_=xv_r[:, sl, :])

        xq_sb = constp.tile([128, NQT, D], F32, name="xq_sb")
        xq_r = xq_d.rearrange("(n p) c -> p n c", p=128)
        nc.sync.dma_start(out=xq_sb[:], in_=xq_r[:])

        w1_sb = constp.tile([128, 2, H], MM_DT, name="w1_sb")
        nc.sync.dma_start(out=w1_sb[:], in_=w1_d.rearrange("(dc p) h -> p dc h", p=128))
        w2_sb = constp.tile([128, 8, D], MM_DT, name="w2_sb")
        nc.sync.dma_start(out=w2_sb[:], in_=w2_d.rearrange("(hc p) d -> p hc d", p=128))
        b1_sb = constp.tile([128, 8], F32, name="b1_sb")
        nc.sync.dma_start(out=b1_sb[:], in_=b1_d.rearrange("(hc p) -> p hc", p=128))
        if use_b2:
            b2_sb = constp.tile([128, D], F32, name="b2_sb")
            nc.sync.dma_start(out=b2_sb[:], in_=bcast_row(b2_d[:]))

        if use_gb1:
            g1_sb = constp.tile([128, D], F32, name="g1_sb")
            nc.sync.dma_start(out=g1_sb[:], in_=bcast_row(g1_d[:]))
            bt1_sb = constp.tile([128, D], F32, name="bt1_sb")
            nc.sync.dma_start(out=bt1_sb[:], in_=bcast_row(bt1_d[:]))
        if use_gb2:
            g2_sb = constp.tile([128, D], F32, name="g2_sb")
            nc.sync.dma_start(out=g2_sb[:], in_=bcast_row(g2_d[:]))
            bt2_sb = constp.tile([128, D], F32, name="bt2_sb")
            nc.sync.dma_start(out=bt2_sb[:], in_=bcast_row(bt2_d[:]))

        ident_sb = constp.tile([128, 128], F32, name="ident_sb")
        make_identity(nc, ident_sb[:])
        eps_sb = constp.tile([128, 1], F32, name="eps_sb")
        nc.vector.memset(eps_sb[:], 1e-5)

        h_all = hallp.tile([128, NQT, D], F32, name="h_all")

        def layer_norm(dst, src, use_gb, g_sb, bt_sb):
            stats = statp.tile([128, 6], F32, name="stats", tag="stats")
            nc.vector.bn_stats(stats[:], src)
            mv = statp.tile([128, 2], F32, name="mv", tag="mv")
            nc.vector.bn_aggr(mv[:], stats[:])
            rstd = statp.tile([128, 1], F32, name="rstd", tag="rstd")
            nc.scalar.activation(rstd[:], mv[:, 1:2], AF.Sqrt, bias=eps_sb[:])
            nc.vector.reciprocal(rstd[:], rstd[:])
            nc.vector.tensor_scalar(
                out=dst,
                in0=src,
                scalar1=mv[:, 0:1],
                scalar2=rstd[:],
                op0=ALU.subtract,
                op1=ALU.mult,
            )
            if use_gb:
                nc.vector.tensor_mul(dst, dst, g_sb[:])
                nc.vector.tensor_add(dst, dst, bt_sb[:])

        for qs in range(NSTRIP):
            q0 = qs * QS

            # ---- scores^T + exp: ST[k-chunk, q-strip] tiles
            pts = []
            for kc in range(NKC):
                stp = ps_st.tile([128, QS], F32, name="stp", tag="stp")
                ks = slice(kc * 128, (kc + 1) * 128)
                nc.tensor.matmul(
                    stp[:], xkT_sb[:, 0, ks], xkT_sb[:, 0, q0 : q0 + QS],
                    start=True, stop=False,
                )
                nc.tensor.matmul(
                    stp[:], xkT_sb[:, 1, ks], xkT_sb[:, 1, q0 : q0 + QS],
                    start=False, stop=True,
                )
                pt = ptp.tile([128, QS], MM_DT, name="pt", tag="pt")
                nc.scalar.activation(pt[:], stp[:], AF.Exp, scale=scale)
                pts.append(pt)

            # ---- PV (+ row-sum via ones column) + normalize + residual + LN1
            for qt in range(QS // 128):
                qg = qs * (QS // 128) + qt
                qsl = slice(qt * 128, (qt + 1) * 128)
                op = ps_o.tile([128, D + 1], F32, name="op", tag="op")
                for kc in range(NKC):
                    nc.tensor.matmul(
                        op[:], pts[kc][:, qsl], xv_sb[:, kc, :],
                        start=(kc == 0), stop=(kc == NKC - 1),
                    )
                recip = statp.tile([128, 1], F32, name="recip", tag="recip")
                nc.vector.reciprocal(recip[:], op[:, D : D + 1])
                r1 = workp.tile([128, D], F32, name="r1", tag="r1")
                # r1 = (attn_unnorm * 1/rowsum) + x_residual, one DVE pass
                nc.vector.scalar_tensor_tensor(
                    out=r1[:], in0=op[:, 0:D], scalar=recip[:],
                    in1=xq_sb[:, qg, :], op0=ALU.mult, op1=ALU.add,
                )
                layer_norm(
                    h_all[:, qg, :], r1[:], use_gb1,
                    g1_sb if use_gb1 else None, bt1_sb if use_gb1 else None,
                )

            # ---- transpose h strip -> hT[d, q]
            ht = htp.tile([128, 2, QS], MM_DT, name="ht", tag="ht")
            for qt in range(QS // 128):
                qg = qs * (QS // 128) + qt
                for dc in range(2):
                    tp = ps_misc.tile([128, 128], F32, name="tp", tag="misc")
                    nc.tensor.transpose(
                        tp[:], h_all[:, qg, dc * 128 : (dc + 1) * 128], ident_sb[:]
                    )
                    nc.scalar.copy(ht[:, dc, qt * 128 : (qt + 1) * 128], tp[:])

            # ---- FFN1: f1T[h, q] = relu(W1^T h^T + b1)
            f1t = f1p.tile([128, 8, QS], MM_DT, name="f1t", tag="f1t")
            for hc in range(8):
                hsl = slice(hc * 128, (hc + 1) * 128)
                fp = ps_f1.tile([128, QS], F32, name="fp", tag="fp")
                nc.tensor.matmul(
                    fp[:], w1_sb[:, 0, hsl], ht[:, 0, :], start=True, stop=False
                )
                nc.tensor.matmul(
                    fp[:], w1_sb[:, 1, hsl], ht[:, 1, :], start=False, stop=True
                )
                nc.scalar.activation(
                    f1t[:, hc, :], fp[:], AF.Relu, bias=b1_sb[:, hc : hc + 1]
                )

            # ---- FFN2 (+ b2) + residual + LN2 -> out
            for qt in range(QS // 128):
                qg = qs * (QS // 128) + qt
                qsl = slice(qt * 128, (qt + 1) * 128)
                f2 = ps_misc.tile([128, D], F32, name="f2", tag="misc")
                for hc in range(8):
                    nc.tensor.matmul(
                        f2[:], f1t[:, hc, qsl], w2_sb[:, hc, :],
                        start=(hc == 0), stop=(hc == 7),
                    )
                r2 = workp.tile([128, D], F32, name="r2", tag="r2")
                nc.vector.tensor_add(r2[:], f2[:], h_all[:, qg, :])
                if use_b2:
                    nc.vector.tensor_add(r2[:], r2[:], b2_sb[:])
                o_t = workp.tile([128, D], F32, name="o_t", tag="o_t")
                layer_norm(
                    o_t[:], r2[:], use_gb2,
                    g2_sb if use_gb2 else None, bt2_sb if use_gb2 else None,
                )
                nc.sync.dma_start(out=out_d[qg * 128 : (qg + 1) * 128, :], in_=o_t[:])

    if not nc.is_finalized():
        nc.finalize()
    return nc


_cache: dict = {}


def _get_program(scale: float, use_gb1: bool, use_gb2: bool, use_b2: bool):
    key = (scale, use_gb1, use_gb2, use_b2)
    if key not in _cache:
        _cache[key] = build_program(scale, use_gb1, use_gb2, use_b2)
    return _cache[key]


def run(inputs: dict, trace: bool = False):
    """Returns (full_output [B,S,D], BassKernelResults)."""
    x = np.ascontiguousarray(np.asarray(inputs["x"], dtype=np.float32))
    W1 = np.asarray(inputs["W1"], dtype=np.float32)
    W2 = np.asarray(inputs["W2"], dtype=np.float32)
    b1 = np.ascontiguousarray(np.asarray(inputs["b1"], dtype=np.float32))
    b2 = np.ascontiguousarray(np.asarray(inputs["b2"], dtype=np.float32))
    gamma1 = np.asarray(inputs["gamma1"], dtype=np.float32)
    beta1 = np.asarray(inputs["beta1"], dtype=np.float32)
    gamma2 = np.asarray(inputs["gamma2"], dtype=np.float32)
    beta2 = np.asarray(inputs["beta2"], dtype=np.float32)
    n_head = int(np.asarray(inputs["n_head"]))
    scale = float((D // n_head) ** -0.5)

    use_gb1 = not (np.all(gamma1 == 1.0) and np.all(beta1 == 0.0))
    use_gb2 = not (np.all(gamma2 == 1.0) and np.all(beta2 == 0.0))
    use_b2 = bool(np.any(b2 != 0.0))

    nc = _get_program(scale, use_gb1, use_gb2, use_b2)

    w1_c = np.ascontiguousarray(W1.astype(MM_NP))
    w2_c = np.ascontiguousarray(W2.astype(MM_NP))

    in_maps = []
    for c in range(NCORES):
        b, half = divmod(c, 2)
        xb = x[b]
        xrot = np.roll(xb, -half * SQ, axis=0) if half else xb
        xkT = np.ascontiguousarray(xrot.T.astype(MM_NP))
        xv = np.empty((S, D + 1), MM_NP)
        xv[:, :D] = xrot.astype(MM_NP)
        xv[:, D] = 1.0
        m = {
            "xkT": xkT,
            "xv": xv,
            "xq": np.ascontiguousarray(xrot[:SQ]),
            "w1": w1_c,
            "w2": w2_c,
            "b1": b1,
        }
        if use_b2:
            m["b2"] = b2
        if use_gb1:
            m["g1"] = gamma1
            m["bt1"] = beta1
        if use_gb2:
            m["g2"] = gamma2
            m["bt2"] = beta2
        in_maps.append(m)

    res = run_bass_kernel_spmd(nc, in_maps, core_ids=list(range(NCORES)), trace=trace)

    out = np.empty((B, S, D), np.float32)
    for c in range(NCORES):
        b, half = divmod(c, 2)
        out[b, half * SQ : (half + 1) * SQ] = res.results[c]["out"]
    return out, res


def kernel(**inputs) -> np.ndarray:
    out, _ = run(inputs)
    return out


# revision 7
# speedup vs baseline: 4566.3146x; 4566.3146x over previous
"""Trainium2 Bass kernel for a cached-encoder-layer block.

Computation (per batch b):
    S  = (x_b @ x_b^T) * scale          # single-head scores, scale=(D//n_head)^-0.5
    P  = softmax(S, axis=-1)
    a  = P @ x_b
    h  = LN(a + x_b) * gamma1 + beta1   # LayerNorm over D
    f  = relu(h @ W1 + b1) @ W2 + b2
    out= LN(f + h) * gamma2 + beta2

Sharding: 8 cores = (batch b in 0..3) x (query-half in 0..1). Each core gets
its batch's keys/values rotated so its 2048 queries are rows 0..2047; softmax
is permutation-invariant over keys so the rotation is harmless and makes the
program identical (SPMD) on all cores.

Per-core kernel strategy:
  - scores are computed TRANSPOSED: ST[k, q] tiles, so that after exp() the
    probability tiles are directly the lhsT (stationary) operand of the PV
    matmul - no on-device transposes in the attention path.
  - softmax is computed without max-subtraction (safe: scores*scale <= ~70
    for randn-scale inputs, well within fp32 exp range); the row-sum comes
    for free as a 257th "ones" column appended to the value matrix.
  - matmul operands are held in MM_DT (bf16 default, fp32r optional); all
    accumulation is fp32 in PSUM; layernorm/softmax statistics are fp32.
  - FFN computes relu(h@W1+b1) transposed (f1T[h, q]) so b1 is a
    per-partition activation bias and f1T feeds FFN2 as lhsT directly.
"""

import os

import ml_dtypes
import numpy as np

import concourse.bacc as bacc
import concourse.bass as bass
import concourse.mybir as mybir
import concourse.tile as tile
from concourse.bass_utils import run_bass_kernel_spmd
from concourse.masks import make_identity

B, S, D, H = 4, 4096, 256, 1024
NCORES = 8
SQ = S // 2            # queries per core
QS = 256               # q-strip width
NSTRIP = SQ // QS      # 8
NKC = S // 128         # 32 key chunks
NQT = SQ // 128        # 16 q tiles per core
F32 = mybir.dt.float32
AF = mybir.ActivationFunctionType
ALU = mybir.AluOpType

if os.environ.get("MM_DT", "bf16") == "f32r":
    MM_DT = mybir.dt.float32r
    MM_NP = np.float32
else:
    MM_DT = mybir.dt.bfloat16
    MM_NP = ml_dtypes.bfloat16


def build_program(scale: float, use_gb1: bool, use_gb2: bool, use_b2: bool,
                  reps: int = 1):
    nc = bacc.Bacc(trn_type="TRN2")

    xkT_d = nc.dram_tensor("xkT", [D, S], MM_DT, kind="ExternalInput")
    xv_d = nc.dram_tensor("xv", [S, D + 1], MM_DT, kind="ExternalInput")
    xq_d = nc.dram_tensor("xq", [SQ, D], F32, kind="ExternalInput")
    w1_d = nc.dram_tensor("w1", [D, H], MM_DT, kind="ExternalInput")
    w2_d = nc.dram_tensor("w2", [H, D], MM_DT, kind="ExternalInput")
    b1_d = nc.dram_tensor("b1", [H], F32, kind="ExternalInput")
    if use_b2:
        b2_d = nc.dram_tensor("b2", [D], F32, kind="ExternalInput")
    if use_gb1:
        g1_d = nc.dram_tensor("g1", [D], F32, kind="ExternalInput")
        bt1_d = nc.dram_tensor("bt1", [D], F32, kind="ExternalInput")
    if use_gb2:
        g2_d = nc.dram_tensor("g2", [D], F32, kind="ExternalInput")
        bt2_d = nc.dram_tensor("bt2", [D], F32, kind="ExternalInput")
    out_d = nc.dram_tensor("out", [SQ, D], F32, kind="ExternalOutput")

    def bcast_row(ap_1d, parts=128):
        # [N] dram vector -> [[0,parts],[1,N]] AP (same row in every partition)
        return bass.AP(
            tensor=ap_1d.tensor, offset=ap_1d.offset, ap=[[0, parts], ap_1d.ap[0]]
        )

    with (
        tile.TileContext(nc) as tc,
        tc.tile_pool(name="const", bufs=1) as constp,
        tc.tile_pool(name="ptp", bufs=44) as ptp,
        tc.tile_pool(name="hall", bufs=1) as hallp,
        tc.tile_pool(name="htp", bufs=3) as htp,
        tc.tile_pool(name="f1p", bufs=2) as f1p,
        tc.tile_pool(name="workp", bufs=4) as workp,
        tc.tile_pool(name="statp", bufs=8) as statp,
        tc.tile_pool(name="ps_st", bufs=2, space="PSUM") as ps_st,
        tc.tile_pool(name="ps_o", bufs=2, space="PSUM") as ps_o,
        tc.tile_pool(name="ps_f1", bufs=2, space="PSUM") as ps_f1,
        tc.tile_pool(name="ps_misc", bufs=2, space="PSUM") as ps_misc,
    ):
        # ---------------- resident inputs ----------------
        xkT_sb = constp.tile([128, 2, S], MM_DT, name="xkT_sb")
        xkT_r = xkT_d.rearrange("(dc p) k -> p dc k", p=128)
        for i in range(8):
            sl = slice(i * (S // 8), (i + 1) * (S // 8))
            nc.sync.dma_start(out=xkT_sb[:, :, sl], in_=xkT_r[:, :, sl])

        xv_sb = constp.tile([128, NKC, D + 1], MM_DT, name="xv_sb")
        xv_r = xv_d.rearrange("(n p) c -> p n c", p=128)
        for i in range(8):
            sl = slice(i * (NKC // 8), (i + 1) * (NKC // 8))
            nc.sync.dma_start(out=xv_sb[:, sl, :], in_=xv_r[:, sl, :])

        xq_sb = constp.tile([128, NQT, D], F32, name="xq_sb")
        xq_r = xq_d.rearrange("(n p) c -> p n c", p=128)
        nc.sync.dma_start(out=xq_sb[:], in_=xq_r[:])

        w1_sb = constp.tile([128, 2, H], MM_DT, name="w1_sb")
        nc.sync.dma_start(out=w1_sb[:], in_=w1_d.rearrange("(dc p) h -> p dc h", p=128))
        w2_sb = constp.tile([128, 8, D], MM_DT, name="w2_sb")
        nc.sync.dma_start(out=w2_sb[:], in_=w2_d.rearrange("(hc p) d -> p hc d", p=128))
        b1_sb = constp.tile([128, 8], F32, name="b1_sb")
        nc.sync.dma_start(out=b1_sb[:], in_=b1_d.rearrange("(hc p) -> p hc", p=128))
        if use_b2:
            b2_sb = constp.tile([128, D], F32, name="b2_sb")
            nc.sync.dma_start(out=b2_sb[:], in_=bcast_row(b2_d[:]))

        if use_gb1:
            g1_sb = constp.tile([128, D], F32, name="g1_sb")
            nc.sync.dma_start(out=g1_sb[:], in_=bcast_row(g1_d[:]))
            bt1_sb = constp.tile([128, D], F32, name="bt1_sb")
            nc.sync.dma_start(out=bt1_sb[:], in_=bcast_row(bt1_d[:]))
        if use_gb2:
            g2_sb = constp.tile([128, D], F32, name="g2_sb")
            nc.sync.dma_start(out=g2_sb[:], in_=bcast_row(g2_d[:]))
            bt2_sb = constp.tile([128, D], F32, name="bt2_sb")
            nc.sync.dma_start(out=bt2_sb[:], in_=bcast_row(bt2_d[:]))

        ident_sb = constp.tile([128, 128], F32, name="ident_sb")
        make_identity(nc, ident_sb[:])
        eps_sb = constp.tile([128, 1], F32, name="eps_sb")
        nc.vector.memset(eps_sb[:], 1e-5)

        h_all = hallp.tile([128, NQT, D], F32, name="h_all")

        def layer_norm(dst, src, use_gb, g_sb, bt_sb):
            stats = statp.tile([128, 6], F32, name="stats", tag="stats")
            nc.vector.bn_stats(stats[:], src)
            mv = statp.tile([128, 2], F32, name="mv", tag="mv")
            nc.vector.bn_aggr(mv[:], stats[:])
            rstd = statp.tile([128, 1], F32, name="rstd", tag="rstd")
            nc.scalar.activation(rstd[:], mv[:, 1:2], AF.Sqrt, bias=eps_sb[:])
            nc.vector.reciprocal(rstd[:], rstd[:])
            nc.vector.tensor_scalar(
                out=dst,
                in0=src,
                scalar1=mv[:, 0:1],
                scalar2=rstd[:],
                op0=ALU.subtract,
                op1=ALU.mult,
            )
            if use_gb:
                nc.vector.tensor_mul(dst, dst, g_sb[:])
                nc.vector.tensor_add(dst, dst, bt_sb[:])

        for qs_rep in range(NSTRIP * reps):
            qs = qs_rep % NSTRIP
            q0 = qs * QS

            # ---- scores^T + exp: ST[k-chunk, q-strip] tiles
            pts = []
            for kc in range(NKC):
                stp = ps_st.tile([128, QS], F32, name="stp", tag="stp")
                ks = slice(kc * 128, (kc + 1) * 128)
                nc.tensor.matmul(
                    stp[:], xkT_sb[:, 0, ks], xkT_sb[:, 0, q0 : q0 + QS],
                    start=True, stop=False,
                )
                nc.tensor.matmul(
                    stp[:], xkT_sb[:, 1, ks], xkT_sb[:, 1, q0 : q0 + QS],
                    start=False, stop=True,
                )
                pt = ptp.tile([128, QS], MM_DT, name="pt", tag="pt")
                nc.scalar.activation(pt[:], stp[:], AF.Exp, scale=scale)
                pts.append(pt)

            # ---- PV (+ row-sum via ones column) + normalize + residual + LN1
            for qt in range(QS // 128):
                qg = qs * (QS // 128) + qt
                qsl = slice(qt * 128, (qt + 1) * 128)
                op = ps_o.tile([128, D + 1], F32, name="op", tag="op")
                for kc in range(NKC):
                    nc.tensor.matmul(
                        op[:], pts[kc][:, qsl], xv_sb[:, kc, :],
                        start=(kc == 0), stop=(kc == NKC - 1),
                    )
                recip = statp.tile([128, 1], F32, name="recip", tag="recip")
                nc.vector.reciprocal(recip[:], op[:, D : D + 1])
                r1 = workp.tile([128, D], F32, name="r1", tag="r1")
                # r1 = (attn_unnorm * 1/rowsum) + x_residual, one DVE pass
                nc.vector.scalar_tensor_tensor(
                    out=r1[:], in0=op[:, 0:D], scalar=recip[:],
                    in1=xq_sb[:, qg, :], op0=ALU.mult, op1=ALU.add,
                )
                layer_norm(
                    h_all[:, qg, :], r1[:], use_gb1,
                    g1_sb if use_gb1 else None, bt1_sb if use_gb1 else None,
                )

            # ---- transpose h strip -> hT[d, q]
            ht = htp.tile([128, 2, QS], MM_DT, name="ht", tag="ht")
            for qt in range(QS // 128):
                qg = qs * (QS // 128) + qt
                for dc in range(2):
                    tp = ps_misc.tile([128, 128], F32, name="tp", tag="misc")
                    nc.tensor.transpose(
                        tp[:], h_all[:, qg, dc * 128 : (dc + 1) * 128], ident_sb[:]
                    )
                    nc.scalar.copy(ht[:, dc, qt * 128 : (qt + 1) * 128], tp[:])

            # ---- FFN1: f1T[h, q] = relu(W1^T h^T + b1)
            f1t = f1p.tile([128, 8, QS], MM_DT, name="f1t", tag="f1t")
            for hc in range(8):
                hsl = slice(hc * 128, (hc + 1) * 128)
                fp = ps_f1.tile([128, QS], F32, name="fp", tag="fp")
                nc.tensor.matmul(
                    fp[:], w1_sb[:, 0, hsl], ht[:, 0, :], start=True, stop=False
                )
                nc.tensor.matmul(
                    fp[:], w1_sb[:, 1, hsl], ht[:, 1, :], start=False, stop=True
                )
                nc.scalar.activation(
                    f1t[:, hc, :], fp[:], AF.Relu, bias=b1_sb[:, hc : hc + 1]
                )

            # ---- FFN2 (+ b2) + residual + LN2 -> out
            for qt in range(QS // 128):
                qg = qs * (QS // 128) + qt
                qsl = slice(qt * 128, (qt + 1) * 128)
                f2 = ps_misc.tile([128, D], F32, name="f2", tag="misc")
                for hc in range(8):
                    nc.tensor.matmul(
                        f2[:], f1t[:, hc, qsl], w2_sb[:, hc, :],
                        start=(hc == 0), stop=(hc == 7),
                    )
                r2 = workp.tile([128, D], F32, name="r2", tag="r2")
                nc.vector.tensor_add(r2[:], f2[:], h_all[:, qg, :])
                if use_b2:
                    nc.vector.tensor_add(r2[:], r2[:], b2_sb[:])
                o_t = workp.tile([128, D], F32, name="o_t", tag="o_t")
                layer_norm(
                    o_t[:], r2[:], use_gb2,
                    g2_sb if use_gb2 else None, bt2_sb if use_gb2 else None,
                )
                nc.sync.dma_start(out=out_d[qg * 128 : (qg + 1) * 128, :], in_=o_t[:])

    if not nc.is_finalized():
        nc.finalize()
    return nc


_cache: dict = {}


def _get_program(scale: float, use_gb1: bool, use_gb2: bool, use_b2: bool):
    key = (scale, use_gb1, use_gb2, use_b2)
    if key not in _cache:
        _cache[key] = build_program(scale, use_gb1, use_gb2, use_b2)
    return _cache[key]


def run(inputs: dict, trace: bool = False):
    """Returns (full_output [B,S,D], BassKernelResults)."""
    x = np.ascontiguousarray(np.asarray(inputs["x"], dtype=np.float32))
    W1 = np.asarray(inputs["W1"], dtype=np.float32)
    W2 = np.asarray(inputs["W2"], dtype=np.float32)
    b1 = np.ascontiguousarray(np.asarray(inputs["b1"], dtype=np.float32))
    b2 = np.ascontiguousarray(np.asarray(inputs["b2"], dtype=np.float32))
    gamma1 = np.asarray(inputs["gamma1"], dtype=np.float32)
    beta1 = np.asarray(inputs["beta1"], dtype=np.float32)
    gamma2 = np.asarray(inputs["gamma2"], dtype=np.float32)
    beta2 = np.asarray(inputs["beta2"], dtype=np.float32)
    n_head = int(np.asarray(inputs["n_head"]))
    scale = float((D // n_head) ** -0.5)

    use_gb1 = not (np.all(gamma1 == 1.0) and np.all(beta1 == 0.0))
    use_gb2 = not (np.all(gamma2 == 1.0) and np.all(beta2 == 0.0))
    use_b2 = bool(np.any(b2 != 0.0))

    nc = _get_program(scale, use_gb1, use_gb2, use_b2)

    w1_c = np.ascontiguousarray(W1.astype(MM_NP))
    w2_c = np.ascontiguousarray(W2.astype(MM_NP))

    in_maps = []
    for c in range(NCORES):
        b, half = divmod(c, 2)
        xb = x[b]
        xrot = np.roll(xb, -half * SQ, axis=0) if half else xb
        xkT = np.ascontiguousarray(xrot.T.astype(MM_NP))
        xv = np.empty((S, D + 1), MM_NP)
        xv[:, :D] = xrot.astype(MM_NP)
        xv[:, D] = 1.0
        m = {
            "xkT": xkT,
            "xv": xv,
            "xq": np.ascontiguousarray(xrot[:SQ]),
            "w1": w1_c,
            "w2": w2_c,
            "b1": b1,
        }
        if use_b2:
            m["b2"] = b2
        if use_gb1:
            m["g1"] = gamma1
            m["bt1"] = beta1
        if use_gb2:
            m["g2"] = gamma2
            m["bt2"] = beta2
        in_maps.append(m)

    global _last_in_maps
    _last_in_maps = in_maps
    res = run_bass_kernel_spmd(nc, in_maps, core_ids=list(range(NCORES)), trace=trace)

    out = np.empty((B, S, D), np.float32)
    for c in range(NCORES):
        b, half = divmod(c, 2)
        out[b, half * SQ : (half + 1) * SQ] = res.results[c]["out"]
    return out, res


def kernel(**inputs) -> np.ndarray:
    out, _ = run(inputs)
    return out


# revision 13
# speedup vs baseline: 14017.5376x; 3.0698x over previous
"""Trainium2 Bass kernel for a cached-encoder-layer block.

Computation (per batch b):
    S  = (x_b @ x_b^T) * scale          # single-head scores, scale=(D//n_head)^-0.5
    P  = softmax(S, axis=-1)
    a  = P @ x_b
    h  = LN(a + x_b) * gamma1 + beta1   # LayerNorm over D
    f  = relu(h @ W1 + b1) @ W2 + b2
    out= LN(f + h) * gamma2 + beta2

Sharding: 8 cores = (batch b in 0..3) x (query-half in 0..1). Each core gets
its batch's keys/values rotated so its 2048 queries are rows 0..2047; softmax
is permutation-invariant over keys so the rotation is harmless and makes the
program identical (SPMD) on all cores.

Per-core kernel strategy:
  - scores are computed TRANSPOSED: ST[k, q] tiles, so that after exp() the
    probability tiles are directly the lhsT (stationary) operand of the PV
    matmul - no on-device transposes in the attention path.
  - softmax is computed without max-subtraction (safe: scores*scale <= ~70
    for randn-scale inputs, well within fp32 exp range); the row-sum comes
    for free as a 257th "ones" column appended to the value matrix.
  - matmul operands are held in MM_DT (bf16 default, fp32r optional); all
    accumulation is fp32 in PSUM; layernorm/softmax statistics are fp32.
  - FFN computes relu(h@W1+b1) transposed (f1T[h, q]) so b1 is a
    per-partition activation bias and f1T feeds FFN2 as lhsT directly.
"""

import os

import ml_dtypes
import numpy as np

import concourse.bacc as bacc
import concourse.bass as bass
import concourse.mybir as mybir
import concourse.tile as tile
from concourse.bass_utils import run_bass_kernel_spmd
from concourse.masks import make_identity

B, S, D, H = 4, 4096, 256, 1024
NCORES = 8
SQ = S // 2            # queries per core
QS = 512               # q-strip width
NSTRIP = SQ // QS      # 8
NKC = S // 128         # 32 key chunks
NQT = SQ // 128        # 16 q tiles per core
F32 = mybir.dt.float32
AF = mybir.ActivationFunctionType
ALU = mybir.AluOpType

if os.environ.get("MM_DT", "bf16") == "f32r":
    MM_DT = mybir.dt.float32r
    MM_NP = np.float32
else:
    MM_DT = mybir.dt.bfloat16
    MM_NP = ml_dtypes.bfloat16


def build_program(scale: float, use_gb1: bool, use_gb2: bool, use_b2: bool,
                  use_b1: bool = True, reps: int = 1):
    nc = bacc.Bacc(trn_type="TRN2")

    xkT_d = nc.dram_tensor("xkT", [D, S], MM_DT, kind="ExternalInput")
    xv_d = nc.dram_tensor("xv", [S, D + 1], MM_DT, kind="ExternalInput")
    xq_d = nc.dram_tensor("xq", [SQ, D], F32, kind="ExternalInput")
    w1_d = nc.dram_tensor("w1", [D, H], MM_DT, kind="ExternalInput")
    w2_d = nc.dram_tensor("w2", [H, D], MM_DT, kind="ExternalInput")
    b1_d = nc.dram_tensor("b1", [H], F32, kind="ExternalInput")
    if use_b2:
        b2_d = nc.dram_tensor("b2", [D], F32, kind="ExternalInput")
    if use_gb1:
        g1_d = nc.dram_tensor("g1", [D], F32, kind="ExternalInput")
        bt1_d = nc.dram_tensor("bt1", [D], F32, kind="ExternalInput")
    if use_gb2:
        g2_d = nc.dram_tensor("g2", [D], F32, kind="ExternalInput")
        bt2_d = nc.dram_tensor("bt2", [D], F32, kind="ExternalInput")
    out_d = nc.dram_tensor("out", [SQ, D], F32, kind="ExternalOutput")

    def bcast_row(ap_1d, parts=128):
        # [N] dram vector -> [[0,parts],[1,N]] AP (same row in every partition)
        return bass.AP(
            tensor=ap_1d.tensor, offset=ap_1d.offset, ap=[[0, parts], ap_1d.ap[0]]
        )

    with (
        tile.TileContext(nc) as tc,
        tc.tile_pool(name="const", bufs=1) as constp,
        tc.tile_pool(name="ptp", bufs=44) as ptp,
        tc.tile_pool(name="hall", bufs=1) as hallp,
        tc.tile_pool(name="htp", bufs=3) as htp,
        tc.tile_pool(name="f1p", bufs=2) as f1p,
        tc.tile_pool(name="workp", bufs=4) as workp,
        tc.tile_pool(name="statp", bufs=8) as statp,
        tc.tile_pool(name="ps_st", bufs=2, space="PSUM") as ps_st,
        tc.tile_pool(name="ps_o", bufs=2, space="PSUM") as ps_o,
        tc.tile_pool(name="ps_f1", bufs=2, space="PSUM") as ps_f1,
        tc.tile_pool(name="ps_misc", bufs=2, space="PSUM") as ps_misc,
    ):
        # ---------------- resident inputs ----------------
        xkT_sb = constp.tile([128, 2, S], MM_DT, name="xkT_sb")
        xkT_r = xkT_d.rearrange("(dc p) k -> p dc k", p=128)
        for i in range(8):
            sl = slice(i * (S // 8), (i + 1) * (S // 8))
            nc.sync.dma_start(out=xkT_sb[:, :, sl], in_=xkT_r[:, :, sl])

        xv_sb = constp.tile([128, NKC, D + 1], MM_DT, name="xv_sb")
        xv_r = xv_d.rearrange("(n p) c -> p n c", p=128)
        for i in range(8):
            sl = slice(i * (NKC // 8), (i + 1) * (NKC // 8))
            nc.sync.dma_start(out=xv_sb[:, sl, :], in_=xv_r[:, sl, :])

        xq_sb = constp.tile([128, NQT, D], F32, name="xq_sb")
        xq_r = xq_d.rearrange("(n p) c -> p n c", p=128)
        nc.sync.dma_start(out=xq_sb[:], in_=xq_r[:])

        w1_sb = constp.tile([128, 2, H], MM_DT, name="w1_sb")
        nc.sync.dma_start(out=w1_sb[:], in_=w1_d.rearrange("(dc p) h -> p dc h", p=128))
        w2_sb = constp.tile([128, 8, D], MM_DT, name="w2_sb")
        nc.sync.dma_start(out=w2_sb[:], in_=w2_d.rearrange("(hc p) d -> p hc d", p=128))
        b1_sb = constp.tile([128, 8], F32, name="b1_sb")
        nc.sync.dma_start(out=b1_sb[:], in_=b1_d.rearrange("(hc p) -> p hc", p=128))
        if use_b2:
            b2_sb = constp.tile([128, D], F32, name="b2_sb")
            nc.sync.dma_start(out=b2_sb[:], in_=bcast_row(b2_d[:]))

        if use_gb1:
            g1_sb = constp.tile([128, D], F32, name="g1_sb")
            nc.sync.dma_start(out=g1_sb[:], in_=bcast_row(g1_d[:]))
            bt1_sb = constp.tile([128, D], F32, name="bt1_sb")
            nc.sync.dma_start(out=bt1_sb[:], in_=bcast_row(bt1_d[:]))
        if use_gb2:
            g2_sb = constp.tile([128, D], F32, name="g2_sb")
            nc.sync.dma_start(out=g2_sb[:], in_=bcast_row(g2_d[:]))
            bt2_sb = constp.tile([128, D], F32, name="bt2_sb")
            nc.sync.dma_start(out=bt2_sb[:], in_=bcast_row(bt2_d[:]))

        ident_sb = constp.tile([128, 128], F32, name="ident_sb")
        make_identity(nc, ident_sb[:])

        h_all = hallp.tile([128, NQT, D], F32, name="h_all")

        NPAIR = QS // 128  # q-tiles per strip

        def ln_stats(src, mv_strip, qt):
            """bn stats for one q-tile into mv_strip[:, qt, :] = (mean, var)."""
            stats = statp.tile([128, 6], F32, name="stats", tag="stats")
            nc.vector.bn_stats(stats[:], src)
            nc.vector.bn_aggr(mv_strip[:, qt, :], stats[:])

        def rsqrt_batch(mv_strip):
            """rstd[:, qt] = 1/sqrt(var_qt + eps) for all q-tiles of a strip,
            entirely on DVE: fast-inverse-sqrt seed + 2 Newton steps."""
            veps = statp.tile([128, NPAIR], F32, name="veps", tag="veps")
            nc.vector.tensor_scalar_add(veps[:], mv_strip[:, :, 1], 1e-5)
            rstd = statp.tile([128, NPAIR], F32, name="rstd", tag="rstd")
            rb = rstd.bitcast(mybir.dt.int32)
            # rb = (veps_bits >> 1) ^ 0xffffffff  ; then += 0x5f3759e0
            # together: rb = 0x5f3759df - (veps_bits >> 1)
            nc.vector.tensor_scalar(
                out=rb[:], in0=veps.bitcast(mybir.dt.int32)[:],
                scalar1=1, scalar2=-1,
                op0=ALU.logical_shift_right, op1=ALU.bitwise_xor,
            )
            nc.vector.tensor_scalar_add(rb[:], rb[:], 0x5F3759E0)
            t = statp.tile([128, NPAIR], F32, name="t", tag="newt")
            for _ in range(2):
                nc.vector.tensor_mul(t[:], rstd[:], rstd[:])
                nc.vector.tensor_mul(t[:], t[:], veps[:])
                nc.vector.tensor_scalar(
                    out=t[:], in0=t[:], scalar1=-0.5, scalar2=1.5,
                    op0=ALU.mult, op1=ALU.add,
                )
                nc.vector.tensor_mul(rstd[:], rstd[:], t[:])
            return rstd

        def ln_apply(dst, src, mv_strip, rstd, qt, use_gb, g_sb, bt_sb):
            nc.vector.tensor_scalar(
                out=dst,
                in0=src,
                scalar1=mv_strip[:, qt, 0:1],
                scalar2=rstd[:, qt : qt + 1],
                op0=ALU.subtract,
                op1=ALU.mult,
            )
            if use_gb:
                nc.vector.tensor_mul(dst, dst, g_sb[:])
                nc.vector.tensor_add(dst, dst, bt_sb[:])

        def emit_strip(qs):
            q0 = qs * QS

            # ---- scores^T + exp: ST[k-chunk, q-strip] tiles
            pts = []
            for kc in range(NKC):
                stp = ps_st.tile([128, QS], F32, name="stp", tag="stp")
                ks = slice(kc * 128, (kc + 1) * 128)
                nc.tensor.matmul(
                    stp[:], xkT_sb[:, 0, ks], xkT_sb[:, 0, q0 : q0 + QS],
                    start=True, stop=False,
                )
                nc.tensor.matmul(
                    stp[:], xkT_sb[:, 1, ks], xkT_sb[:, 1, q0 : q0 + QS],
                    start=False, stop=True,
                )
                pt = ptp.tile([128, QS], MM_DT, name="pt", tag="pt")
                nc.scalar.activation(pt[:], stp[:], AF.Exp, scale=scale)
                pts.append(pt)

            # ---- PV (+ row-sum via ones column) + normalize + residual + LN1
            mv1 = statp.tile([128, NPAIR, 2], F32, name="mv1", tag="mv1")
            r1s = []
            for qt in range(NPAIR):
                qg = qs * NPAIR + qt
                qsl = slice(qt * 128, (qt + 1) * 128)
                op = ps_o.tile([128, D + 1], F32, name="op", tag="op")
                for kc in range(NKC):
                    nc.tensor.matmul(
                        op[:], pts[kc][:, qsl], xv_sb[:, kc, :],
                        start=(kc == 0), stop=(kc == NKC - 1),
                    )
                recip = statp.tile([128, 1], F32, name="recip", tag="recip")
                nc.vector.reciprocal(recip[:], op[:, D : D + 1])
                r1 = workp.tile([128, D], F32, name="r1", tag="r1")
                # r1 = (attn_unnorm * 1/rowsum) + x_residual, one DVE pass
                nc.vector.scalar_tensor_tensor(
                    out=r1[:], in0=op[:, 0:D], scalar=recip[:],
                    in1=xq_sb[:, qg, :], op0=ALU.mult, op1=ALU.add,
                )
                ln_stats(r1[:], mv1, qt)
                r1s.append(r1)
            rstd1 = rsqrt_batch(mv1)
            for qt in range(NPAIR):
                qg = qs * NPAIR + qt
                ln_apply(
                    h_all[:, qg, :], r1s[qt][:], mv1, rstd1, qt, use_gb1,
                    g1_sb if use_gb1 else None, bt1_sb if use_gb1 else None,
                )

            # ---- transpose h strip -> hT[d, q]
            ht = htp.tile([128, 2, QS], MM_DT, name="ht", tag="ht")
            for qt in range(NPAIR):
                qg = qs * NPAIR + qt
                for dc in range(2):
                    tp = ps_misc.tile([128, 128], F32, name="tp", tag="misc")
                    nc.tensor.transpose(
                        tp[:], h_all[:, qg, dc * 128 : (dc + 1) * 128], ident_sb[:]
                    )
                    nc.vector.tensor_copy(ht[:, dc, qt * 128 : (qt + 1) * 128], tp[:])

            # ---- FFN1: f1T[h, q] = relu(W1^T h^T + b1)  (relu on DVE)
            f1t = f1p.tile([128, 8, QS], MM_DT, name="f1t", tag="f1t")
            for hc in range(8):
                hsl = slice(hc * 128, (hc + 1) * 128)
                fp = ps_f1.tile([128, QS], F32, name="fp", tag="fp")
                nc.tensor.matmul(
                    fp[:], w1_sb[:, 0, hsl], ht[:, 0, :], start=True, stop=False
                )
                nc.tensor.matmul(
                    fp[:], w1_sb[:, 1, hsl], ht[:, 1, :], start=False, stop=True
                )
                if use_b1:
                    nc.vector.tensor_scalar(
                        out=f1t[:, hc, :], in0=fp[:],
                        scalar1=b1_sb[:, hc : hc + 1], scalar2=0.0,
                        op0=ALU.add, op1=ALU.max,
                    )
                else:
                    nc.vector.tensor_scalar_max(f1t[:, hc, :], fp[:], 0.0)

            # ---- FFN2 (+ b2) + residual + LN2 -> out
            mv2 = statp.tile([128, NPAIR, 2], F32, name="mv2", tag="mv2")
            r2s = []
            for qt in range(NPAIR):
                qg = qs * NPAIR + qt
                qsl = slice(qt * 128, (qt + 1) * 128)
                f2 = ps_misc.tile([128, D], F32, name="f2", tag="misc")
                for hc in range(8):
                    nc.tensor.matmul(
                        f2[:], f1t[:, hc, qsl], w2_sb[:, hc, :],
                        start=(hc == 0), stop=(hc == 7),
                    )
                r2 = workp.tile([128, D], F32, name="r2", tag="r2")
                nc.vector.tensor_add(r2[:], f2[:], h_all[:, qg, :])
                if use_b2:
                    nc.vector.tensor_add(r2[:], r2[:], b2_sb[:])
                ln_stats(r2[:], mv2, qt)
                r2s.append(r2)
            rstd2 = rsqrt_batch(mv2)
            for qt in range(NPAIR):
                qg = qs * NPAIR + qt
                o_t = workp.tile([128, D], F32, name="o_t", tag="o_t")
                ln_apply(
                    o_t[:], r2s[qt][:], mv2, rstd2, qt, use_gb2,
                    g2_sb if use_gb2 else None, bt2_sb if use_gb2 else None,
                )
                nc.sync.dma_start(out=out_d[qg * 128 : (qg + 1) * 128, :], in_=o_t[:])

        if reps == 1:
            for qs in range(NSTRIP):
                emit_strip(qs)
        else:
            # hardware loop around the whole compute body (for benchmarking:
            # constant instruction count, arbitrary trip count)
            with tc.For_i(0, reps, 1):
                for qs in range(NSTRIP):
                    emit_strip(qs)

    if not nc.is_finalized():
        nc.finalize()
    return nc


_cache: dict = {}


def _get_program(scale: float, use_gb1: bool, use_gb2: bool, use_b2: bool,
                 use_b1: bool):
    key = (scale, use_gb1, use_gb2, use_b2, use_b1)
    if key not in _cache:
        _cache[key] = build_program(scale, use_gb1, use_gb2, use_b2, use_b1)
    return _cache[key]


def run(inputs: dict, trace: bool = False):
    """Returns (full_output [B,S,D], BassKernelResults)."""
    x = np.ascontiguousarray(np.asarray(inputs["x"], dtype=np.float32))
    W1 = np.asarray(inputs["W1"], dtype=np.float32)
    W2 = np.asarray(inputs["W2"], dtype=np.float32)
    b1 = np.ascontiguousarray(np.asarray(inputs["b1"], dtype=np.float32))
    b2 = np.ascontiguousarray(np.asarray(inputs["b2"], dtype=np.float32))
    gamma1 = np.asarray(inputs["gamma1"], dtype=np.float32)
    beta1 = np.asarray(inputs["beta1"], dtype=np.float32)
    gamma2 = np.asarray(inputs["gamma2"], dtype=np.float32)
    beta2 = np.asarray(inputs["beta2"], dtype=np.float32)
    n_head = int(np.asarray(inputs["n_head"]))
    scale = float((D // n_head) ** -0.5)

    use_gb1 = not (np.all(gamma1 == 1.0) and np.all(beta1 == 0.0))
    use_gb2 = not (np.all(gamma2 == 1.0) and np.all(beta2 == 0.0))
    use_b2 = bool(np.any(b2 != 0.0))
    use_b1 = bool(np.any(b1 != 0.0))

    nc = _get_program(scale, use_gb1, use_gb2, use_b2, use_b1)

    w1_c = np.ascontiguousarray(W1.astype(MM_NP))
    w2_c = np.ascontiguousarray(W2.astype(MM_NP))

    in_maps = []
    for c in range(NCORES):
        b, half = divmod(c, 2)
        xb = x[b]
        xrot = np.roll(xb, -half * SQ, axis=0) if half else xb
        xkT = np.ascontiguousarray(xrot.T.astype(MM_NP))
        xv = np.empty((S, D + 1), MM_NP)
        xv[:, :D] = xrot.astype(MM_NP)
        xv[:, D] = 1.0
        m = {
            "xkT": xkT,
            "xv": xv,
            "xq": np.ascontiguousarray(xrot[:SQ]),
            "w1": w1_c,
            "w2": w2_c,
            "b1": b1,
        }
        if use_b2:
            m["b2"] = b2
        if use_gb1:
            m["g1"] = gamma1
            m["bt1"] = beta1
        if use_gb2:
            m["g2"] = gamma2
            m["bt2"] = beta2
        in_maps.append(m)

    global _last_in_maps
    _last_in_maps = in_maps
    res = run_bass_kernel_spmd(nc, in_maps, core_ids=list(range(NCORES)), trace=trace)

    out = np.empty((B, S, D), np.float32)
    for c in range(NCORES):
        b, half = divmod(c, 2)
        out[b, half * SQ : (half + 1) * SQ] = res.results[c]["out"]
    return out, res


def kernel(**inputs) -> np.ndarray:
    out, _ = run(inputs)
    return out
